# Initial kernel scaffold
#
"""ConceptContrastiveLoss Trainium2 kernel (8-core SPMD, batch-parallel).

Takes FULL inputs expert_concepts/violator_concepts [256, 2048, 128] f32,
returns the scalar loss. Internally shards the batch dim across 8 cores.
Per core (64 MiB of input): each batch item [2048, 128] is DMA'd as one
contiguous 1 MiB transfer into SBUF [128 partitions x 2048] (16 seq rows
per partition), tree-halved on VectorE (exact fp32 adds) down to
[128, 128], then one fp32 ones-matmul on TensorE folds the partitions
into a D-major centroid column accumulated in PSUM. The tiny [128, 64]
centroid block is AllGathered; every core then computes the pairwise
separation/clustering terms redundantly (-2*X^T*Y matmuls plus a rank-2
norms matmul, ACT sqrt/relu/square with accum_out reductions) and core 0's
scalar is returned.

Measured on trn2 (8 cores, via in-kernel For_i repeat loops at R=49 to
cancel the ~92ms axon dispatch overhead): bulk phase ~160-170us per
iteration, against a minimal-compute DMA floor probe of ~145-152us.
Config sweeps all land within noise of that floor: dve_stop 128 vs 512
(~165 both), bpd 1 vs 4 (~165 vs ~176), so the kernel is memory-bound.
The fp32 TensorE-only variant (REDUCE_MODE="pe") is ~325us/iteration:
fp32 matmuls are LoadStationary-bound (~317ns per [128,128] tile) and
weight loads do not lift the PE HAM clock gate. The full single-shot
kernel adds the AllGather + pairwise phase + kernel drain (~35us) on
top of one bulk iteration => ~200us end-to-end estimate, vs a
512MiB/(8 x 358GB/s) = 187us naive HBM roofline.
"""

import numpy as np

import concourse.bacc as bacc
import concourse.bass as bass
import concourse.mybir as mybir
import concourse.tile as tile
from concourse.bass_utils import run_bass_kernel_spmd
from concourse.tile import add_dep_helper

F32 = mybir.dt.float32

MARGIN = 10.0
ALPHA = 3.0
BETA = 0.3
GAMMA = 0.3

B, S, D = 256, 2048, 128
N_CORES = 8
BPD = 1   # batch items per DMA (DMA size = BPD MiB)
BUFS = 8  # big-tile pool buffers
REDUCE_MODE = "dve"  # "pe": 16 ones-matmuls/batch; "dve": tree-halve + matmuls
DVE_STOP = 128       # halving-tree handoff width (elems); PE folds the rest
N_DMA_ENG = 2        # DMA issue streams: 2 = SP+ACT HWDGE, 3 = + gpsimd SWDGE


def _build_body(tc, e, v, out, loc_cent, gath, B, S, D, n_cores, chain=None,
                solo=False, bpd=BPD, bufs=BUFS, loop_r=1, pe_stride=1,
                reduce_mode="pe", dve_stop=None, n_dma_eng=None):
    nc = tc.nc
    if dve_stop is None:
        dve_stop = DVE_STOP
    if n_dma_eng is None:
        n_dma_eng = N_DMA_ENG
    Bl = B // n_cores  # local batches per tensor
    J = S // 128       # seq tiles per batch item
    n_pairs = B * (B - 1) // 2
    w_ev = ALPHA / (B * B)
    w_ee = BETA / (2.0 * n_pairs)
    w_vv = GAMMA / (2.0 * n_pairs)
    blks = [(s, min(128, B - s)) for s in range(0, B, 128)]
    n_blk = len(blks)

    from contextlib import ExitStack

    with ExitStack() as ctx:
        consts = ctx.enter_context(tc.tile_pool(name="consts", bufs=1))
        # preamble-initialized const (no Tile dep => no extra sem wait on
        # matmuls; walrus allows only 1 sync wait per fp32 matmul)
        ones_col = nc.const_aps.aps[(F32, 1.0)]
        b_eps = consts.tile([128, 1], F32, name="b_eps")
        nc.vector.memset(b_eps[:], 1e-12)
        b_margin = consts.tile([128, 1], F32, name="b_margin")
        nc.vector.memset(b_margin[:], MARGIN)
        centS = consts.tile([D, 2 * Bl], F32, name="centS")

        # ---- bulk phase: per-batch centroid sums via TensorE ----
        # 8 batch columns share one PSUM bank (one accumulation group), so
        # all 8 pool bufs are used exactly once -- no slot reuse, and every
        # matmul carries at most the single DMA wait walrus permits.
        cent_copies = []
        GRP = min(8, 2 * Bl)
        n_groups = (2 * Bl + GRP - 1) // GRP
        assert GRP % bpd == 0 and Bl % bpd == 0
        with (
            tc.tile_pool(name="big", bufs=bufs) as big_pool,
            tc.tile_pool(name="cps", bufs=8, space="PSUM") as cps,
        ):
            dma_engines = [nc.sync, nc.scalar, nc.gpsimd][:n_dma_eng]

            def emit_bulk():
                dma_i = 0
                for g in range(n_groups):
                    G = cps.tile([128, 512], F32, name="Gacc")
                    start_mm = None
                    col_last = []
                    for ti in range(GRP // bpd):
                        gi0 = g * GRP + ti * bpd
                        t_idx, b0 = divmod(gi0, Bl)
                        src = (e, v)[t_idx]
                        Tb = big_pool.tile([128, bpd * J * D], F32, name="Tb")
                        eng = dma_engines[dma_i % len(dma_engines)]
                        dma_i += 1
                        if bpd == 1:
                            eng.dma_start(
                                out=Tb[:],
                                in_=src[b0].rearrange("(p j) d -> p (j d)", p=128),
                            )
                        else:
                            eng.dma_start(
                                out=Tb[:].rearrange("p (b x) -> p b x", b=bpd),
                                in_=src[b0 : b0 + bpd].rearrange(
                                    "b (p j) d -> p b (j d)", p=128
                                ),
                            )
                        if reduce_mode == "dve":
                            # tree-halve the 16 seq rows per partition on DVE
                            # (exact fp32 adds) down to width dve_stop --
                            # one 3D-AP op covers all bpd batches in the tile
                            # to amortize DVE instruction overhead
                            w = J * D // 2
                            Tb3 = (
                                Tb[:].rearrange("p (b x) -> p b x", b=bpd)
                                if bpd > 1
                                else None
                            )
                            while w >= dve_stop:
                                if bpd == 1:
                                    nc.vector.tensor_add(
                                        Tb[:, 0:w],
                                        Tb[:, 0:w],
                                        Tb[:, w : 2 * w],
                                    )
                                else:
                                    nc.vector.tensor_add(
                                        Tb3[:, :, 0:w],
                                        Tb3[:, :, 0:w],
                                        Tb3[:, :, w : 2 * w],
                                    )
                                w //= 2
                        for bi in range(bpd):
                            c = ti * bpd + bi
                            if reduce_mode == "dve":
                                base = bi * J * D
                                n_folds = dve_stop // D
                                for fi in range(n_folds):
                                    is_first = c == 0 and fi == 0
                                    is_last = c == GRP - 1 and fi == n_folds - 1
                                    o = base + fi * D
                                    mm = nc.tensor.matmul(
                                        out=G[:, c : c + 1],
                                        lhsT=Tb[:, o : o + D],
                                        rhs=ones_col,
                                        start=is_first,
                                        stop=is_last,
                                    )
                                    if start_mm is None:
                                        start_mm = mm
                                    elif fi == 0:
                                        add_dep_helper(
                                            mm.ins, start_mm.ins, sync=False,
                                            reason="psum group start first",
                                        )
                                    if fi == n_folds - 1:
                                        col_last.append(mm)
                                    if is_last:
                                        for prev in col_last[:-1]:
                                            add_dep_helper(
                                                mm.ins, prev.ins, sync=False,
                                                reason="psum group stop last",
                                            )
                                continue
                            for j in range(0, J, pe_stride):
                                is_last = c == GRP - 1 and j + pe_stride >= J
                                o = (bi * J + j) * D
                                mm = nc.tensor.matmul(
                                    out=G[:, c : c + 1],
                                    lhsT=Tb[:, o : o + D],
                                    rhs=ones_col,
                                    start=(c == 0 and j == 0),
                                    stop=is_last,
                                )
                                if start_mm is None:
                                    start_mm = mm
                                elif j == 0:
                                    add_dep_helper(
                                        mm.ins, start_mm.ins, sync=False,
                                        reason="psum group start first",
                                    )
                                if j == J - 1:
                                    col_last.append(mm)
                                if is_last:
                                    for prev in col_last[:-1]:
                                        add_dep_helper(
                                            mm.ins, prev.ins, sync=False,
                                            reason="psum group stop last",
                                        )
                        start_mm = start_mm
                    cent_copies.append(
                        nc.scalar.mul(
                            centS[:, g * GRP : (g + 1) * GRP], G[:, 0:GRP], 1.0 / S
                        )
                    )

            if loop_r > 1:
                with tc.For_i(0, loop_r, 1) as _i:
                    emit_bulk()
            else:
                emit_bulk()

        # ---- gather centroids across cores ----
        nc.sync.dma_start(out=loc_cent[:], in_=centS[:])
        if solo:
            nc.sync.dma_start(out=gath[:], in_=loc_cent[:])
        else:
            nc.gpsimd.collective_compute(
                "AllGather",
                mybir.AluOpType.bypass,
                replica_groups=[list(range(n_cores))],
                ins=[loc_cent[:]],
                outs=[gath[:]],
            )
        CtEV = consts.tile([D, 2 * B], F32, name="CtEV")
        ct_dma = nc.sync.dma_start(
            out=CtEV[:].rearrange("p (t c j) -> p t c j", t=2, c=n_cores),
            in_=gath.rearrange("(c p) (t j) -> p t c j", c=n_cores, t=2),
        )
        CtE = CtEV[:, 0:B]
        CtV = CtEV[:, B : 2 * B]

        # ---- small pairwise phase (identical on every core) ----
        # PE gate: absorbs the cross-engine deps (cent copies on ACT, the
        # centroid reload DMA) so each later fp32 matmul needs <=1 sem wait
        # (walrus allows only one sync wait on an fp32 matmul's ldweights).
        pe_gate = nc.tensor.nop()
        for ins in cent_copies:
            add_dep_helper(pe_gate.ins, ins.ins, sync=True, reason="pe gate")
        add_dep_helper(pe_gate.ins, ct_dma.ins, sync=True, reason="pe gate dma")
        sp = ctx.enter_context(tc.tile_pool(name="sp", bufs=1))
        m2E = sp.tile([D, B], F32, name="m2E")
        m2V = sp.tile([D, B], F32, name="m2V")
        sqE = sp.tile([D, B], F32, name="sqE")
        sqV = sp.tile([D, B], F32, name="sqV")
        nc.vector.tensor_scalar_mul(m2E[:], CtE, -2.0)
        nc.vector.tensor_scalar_mul(m2V[:], CtV, -2.0)
        nc.vector.tensor_mul(sqE[:], CtE, CtE)
        nc.vector.tensor_mul(sqV[:], CtV, CtV)

        # aug tiles: row 0 and row 32 carry {norms, ones}; rest zero (K=64)
        ag_e = sp.tile([64, B], F32, name="ag_e")    # lhsT rows: n_e, 1
        ag_v = sp.tile([64, B], F32, name="ag_v")    # lhsT rows: n_v, 1
        rhs_e = sp.tile([64, B], F32, name="rhs_e")  # rhs rows: 1, n_e
        rhs_v = sp.tile([64, B], F32, name="rhs_v")  # rhs rows: 1, n_v
        acc = sp.tile([128, 3 * n_blk], F32, name="acc")  # per-partition partials
        nc.vector.memset(acc[:], 0.0)

        with tc.tile_pool(name="sps", bufs=1, space="PSUM") as sps, tc.tile_pool(
            name="spp", bufs=3 * n_blk, space="PSUM"
        ) as spp, tc.tile_pool(name="spf", bufs=1, space="PSUM") as spf, tc.tile_pool(
            name="trash", bufs=2
        ) as trash_pool:
            def pe_mm(*args, **kwargs):
                mm = nc.tensor.matmul(*args, **kwargs)
                add_dep_helper(mm.ins, pe_gate.ins, sync=False, reason="after gate")
                return mm

            psn = sps.tile([128, 512], F32, name="psn")
            # squared norms at partitions 0 and 32
            pe_mm(out=psn[0:1, 0:B], lhsT=ones_col, rhs=sqE[:])
            pe_mm(out=psn[32:33, 0:B], lhsT=ones_col, rhs=sqV[:])
            pe_mm(out=psn[0:1, B : 2 * B], lhsT=ones_col, rhs=sqV[:])
            pe_mm(out=psn[32:33, B : 2 * B], lhsT=ones_col, rhs=sqE[:])

            for t, row0_src, row32_src in (
                (ag_e, psn[0:1, 0:B], None),
                (ag_v, psn[0:1, B : 2 * B], None),
                (rhs_e, None, psn[32:33, B : 2 * B]),
                (rhs_v, None, psn[32:33, 0:B]),
            ):
                nc.vector.memset(t[:], 0.0)
                if row0_src is not None:
                    nc.vector.tensor_copy(t[0:1, :], row0_src)
                    nc.vector.memset(t[32:33, :], 1.0)
                else:
                    nc.vector.memset(t[0:1, :], 1.0)
                    nc.vector.tensor_copy(t[32:33, :], row32_src)

            # EV separation: sq = -2 E^T V + (n_e + n_v), then hinge
            for bi, (bs, bn) in enumerate(blks):
                P_evb = spp.tile([128, 512], F32, name="P_ev")
                P_ev = P_evb[:, 0:B]
                pe_mm(
                    out=P_ev[:bn],
                    lhsT=m2E[:, bs : bs + bn],
                    rhs=CtV,
                    start=True,
                    stop=False,
                )
                pe_mm(
                    out=P_ev[:bn],
                    lhsT=ag_e[:, bs : bs + bn],
                    rhs=rhs_v[:],
                    start=False,
                    stop=True,
                )
                dist = trash_pool.tile([128, B], F32, name="dist")
                hin = trash_pool.tile([128, B], F32, name="hin")
                hsq = trash_pool.tile([128, B], F32, name="hsq")
                nc.vector.tensor_scalar_max(P_ev[:bn], P_ev[:bn], 0.0)
                nc.scalar.activation(
                    dist[:bn], P_ev[:bn], mybir.ActivationFunctionType.Sqrt,
                    bias=b_eps[:bn],
                )
                nc.scalar.activation(
                    hin[:bn],
                    dist[:bn],
                    mybir.ActivationFunctionType.Relu,
                    bias=b_margin[:bn],
                    scale=-1.0,
                )
                nc.scalar.activation(
                    hsq[:bn],
                    hin[:bn],
                    mybir.ActivationFunctionType.Square,
                    accum_out=acc[:bn, bi : bi + 1],
                )

            # EE / VV clustering: sq = -2 X^T X + (n + n), relu, sum
            for pi, (m2, Ct, ag, rhs_t) in enumerate(
                ((m2E, CtE, ag_e, rhs_e), (m2V, CtV, ag_v, rhs_v))
            ):
                for bi, (bs, bn) in enumerate(blks):
                    P_clb = spp.tile([128, 512], F32, name="P_cl", tag="P_ev")
                    P_cl = P_clb[:, 0:B]
                    pe_mm(
                        out=P_cl[:bn],
                        lhsT=m2[:, bs : bs + bn],
                        rhs=Ct,
                        start=True,
                        stop=False,
                    )
                    pe_mm(
                        out=P_cl[:bn],
                        lhsT=ag[:, bs : bs + bn],
                        rhs=rhs_t[:],
                        start=False,
                        stop=True,
                    )
                    rel = trash_pool.tile([128, B], F32, name="rel", tag="dist")
                    ci = (1 + pi) * n_blk + bi
                    nc.scalar.activation(
                        rel[:bn],
                        P_cl[:bn],
                        mybir.ActivationFunctionType.Relu,
                        accum_out=acc[:bn, ci : ci + 1],
                    )

            # combine: tot = w_ev*sum(ev) + w_ee*sum(ee) + w_vv*sum(vv)
            t_ev = sp.tile([128, 1], F32, name="t_ev")
            t_ee = sp.tile([128, 1], F32, name="t_ee")
            t_vv = sp.tile([128, 1], F32, name="t_vv")
            tot = sp.tile([128, 1], F32, name="tot")
            for t, base in ((t_ev, 0), (t_ee, n_blk), (t_vv, 2 * n_blk)):
                if n_blk == 1:
                    nc.vector.tensor_copy(t[:], acc[:, base : base + 1])
                else:
                    nc.vector.tensor_add(
                        t[:], acc[:, base : base + 1], acc[:, base + 1 : base + 2]
                    )
                    for k in range(2, n_blk):
                        nc.vector.tensor_add(t[:], t[:], acc[:, base + k : base + k + 1])
            nc.vector.tensor_scalar_mul(tot[:], t_ev[:], w_ev)
            nc.vector.scalar_tensor_tensor(
                tot[:], t_ee[:], w_ee, tot[:],
                op0=mybir.AluOpType.mult, op1=mybir.AluOpType.add,
            )
            nc.vector.scalar_tensor_tensor(
                tot[:], t_vv[:], w_vv, tot[:],
                op0=mybir.AluOpType.mult, op1=mybir.AluOpType.add,
            )
            psFb = spf.tile([128, 512], F32, name="psF")
            psF = psFb[0:1, 0:1]
            pe_mm(out=psF, lhsT=ones_col, rhs=tot[:])
            fin = sp.tile([1, 1], F32, name="fin")
            if chain is not None:
                ch = sp.tile([1, 1], F32, name="ch")
                nc.sync.dma_start(out=ch[:], in_=chain[:])
                nc.vector.scalar_tensor_tensor(
                    fin[:], ch[:], 0.0, psF,
                    op0=mybir.AluOpType.mult, op1=mybir.AluOpType.add,
                )
            else:
                nc.scalar.copy(fin[:], psF)
            nc.sync.dma_start(out=out[:], in_=fin[:])


def build_nc(B=B, S=S, D=D, n_cores=N_CORES, with_chain=False, solo=False,
             bpd=None, bufs=None, loop_r=1, pe_stride=1, reduce_mode=None,
             dve_stop=None, n_dma_eng=None):
    Bl = B // n_cores
    nc = bacc.Bacc("TRN2", num_devices=n_cores)
    e = nc.dram_tensor("expert_concepts", [Bl, S, D], F32, kind="ExternalInput").ap()
    v = nc.dram_tensor("violator_concepts", [Bl, S, D], F32, kind="ExternalInput").ap()
    chain = (
        nc.dram_tensor("chain", [1, 1], F32, kind="ExternalInput").ap()
        if with_chain
        else None
    )
    out = nc.dram_tensor("out", [1, 1], F32, kind="ExternalOutput").ap()
    loc_cent = nc.dram_tensor("loc_cent", [D, 2 * Bl], F32).ap()
    gath_space = "Local" if solo else "Shared"
    gath = nc.dram_tensor(
        "gath", [n_cores * D, 2 * Bl], F32, addr_space=gath_space
    ).ap()
    with tile.TileContext(nc) as tc:
        _build_body(
            tc, e, v, out, loc_cent, gath, B, S, D, n_cores, chain=chain, solo=solo,
            bpd=bpd if bpd is not None else BPD,
            bufs=bufs if bufs is not None else BUFS,
            loop_r=loop_r,
            pe_stride=pe_stride,
            reduce_mode=reduce_mode if reduce_mode is not None else REDUCE_MODE,
            dve_stop=dve_stop if dve_stop is not None else DVE_STOP,
            n_dma_eng=n_dma_eng,
        )
    nc.compile()
    return nc


def _run(expert_concepts, violator_concepts, **spmd_kwargs):
    expert_concepts = np.ascontiguousarray(expert_concepts, dtype=np.float32)
    violator_concepts = np.ascontiguousarray(violator_concepts, dtype=np.float32)
    assert expert_concepts.shape == (B, S, D)
    assert violator_concepts.shape == (B, S, D)

    nc = build_nc()
    Bl = B // N_CORES
    in_maps = [
        {
            "expert_concepts": expert_concepts[c * Bl : (c + 1) * Bl],
            "violator_concepts": violator_concepts[c * Bl : (c + 1) * Bl],
        }
        for c in range(N_CORES)
    ]
    res = run_bass_kernel_spmd(nc, in_maps, list(range(N_CORES)), **spmd_kwargs)
    return np.float32(res.results[0]["out"][0, 0]), res


def kernel(expert_concepts: np.ndarray, violator_concepts: np.ndarray) -> np.ndarray:
    out, _ = _run(expert_concepts, violator_concepts)
    return out



# revision 1
# speedup vs baseline: 6.0508x; 6.0508x over previous
"""ConceptContrastiveLoss Trainium2 kernel (8-core SPMD, batch-parallel).

Takes FULL inputs expert_concepts/violator_concepts [256, 2048, 128] f32,
returns the scalar loss. Internally shards the batch dim across 8 cores.
Per core (64 MiB of input): each batch item [2048, 128] is DMA'd as one
contiguous 1 MiB transfer into SBUF [128 partitions x 2048] (16 seq rows
per partition), tree-halved on VectorE (exact fp32 adds) down to
[128, 128], then one fp32 ones-matmul on TensorE folds the partitions
into a D-major centroid column accumulated in PSUM. The tiny [128, 64]
centroid block is AllGathered; every core then computes the pairwise
separation/clustering terms redundantly (-2*X^T*Y matmuls plus a rank-2
norms matmul, ACT sqrt/relu/square with accum_out reductions) and core 0's
scalar is returned.

Measured on trn2 (8 cores, via in-kernel For_i repeat loops at R=49 to
cancel the ~92ms axon dispatch overhead): bulk phase ~160-170us per
iteration, against a minimal-compute DMA floor probe of ~145-152us.
Config sweeps all land within noise of that floor: dve_stop 128 vs 512
(~165 both), bpd 1 vs 4 (~165 vs ~176), so the kernel is memory-bound.
The fp32 TensorE-only variant (REDUCE_MODE="pe") is ~325us/iteration:
fp32 matmuls are LoadStationary-bound (~317ns per [128,128] tile) and
weight loads do not lift the PE HAM clock gate. The full single-shot
kernel adds the AllGather + pairwise phase + kernel drain (~35us) on
top of one bulk iteration => ~200us end-to-end estimate, vs a
512MiB/(8 x 358GB/s) = 187us naive HBM roofline.
"""

import numpy as np

import concourse.bacc as bacc
import concourse.bass as bass
import concourse.mybir as mybir
import concourse.tile as tile
from concourse.bass_utils import run_bass_kernel_spmd
from concourse.tile import add_dep_helper

F32 = mybir.dt.float32

MARGIN = 10.0
ALPHA = 3.0
BETA = 0.3
GAMMA = 0.3

B, S, D = 256, 2048, 128
N_CORES = 8
BPD = 1   # batch items per DMA (DMA size = BPD MiB)
BUFS = 8  # big-tile pool buffers
REDUCE_MODE = "dve"  # "pe": 16 ones-matmuls/batch; "dve": tree-halve + matmuls
DVE_STOP = 128       # halving-tree handoff width (elems); PE folds the rest
N_DMA_ENG = 2        # DMA issue streams: 2 = SP+ACT HWDGE, 3 = + gpsimd SWDGE


def _build_body(tc, e, v, out, loc_cent, gath, B, S, D, n_cores, chain=None,
                solo=False, bpd=BPD, bufs=BUFS, loop_r=1, pe_stride=1,
                reduce_mode="pe", dve_stop=None, n_dma_eng=None):
    nc = tc.nc
    if dve_stop is None:
        dve_stop = DVE_STOP
    if n_dma_eng is None:
        n_dma_eng = N_DMA_ENG
    Bl = B // n_cores  # local batches per tensor
    J = S // 128       # seq tiles per batch item
    n_pairs = B * (B - 1) // 2
    w_ev = ALPHA / (B * B)
    w_ee = BETA / (2.0 * n_pairs)
    w_vv = GAMMA / (2.0 * n_pairs)
    blks = [(s, min(128, B - s)) for s in range(0, B, 128)]
    n_blk = len(blks)

    from contextlib import ExitStack

    with ExitStack() as ctx:
        consts = ctx.enter_context(tc.tile_pool(name="consts", bufs=1))
        # preamble-initialized const (no Tile dep => no extra sem wait on
        # matmuls; walrus allows only 1 sync wait per fp32 matmul)
        ones_col = nc.const_aps.aps[(F32, 1.0)]
        b_eps = consts.tile([128, 1], F32, name="b_eps")
        nc.vector.memset(b_eps[:], 1e-12)
        b_margin = consts.tile([128, 1], F32, name="b_margin")
        nc.vector.memset(b_margin[:], MARGIN)
        centS = consts.tile([D, 2 * Bl], F32, name="centS")

        # ---- bulk phase: per-batch centroid sums via TensorE ----
        # 8 batch columns share one PSUM bank (one accumulation group), so
        # all 8 pool bufs are used exactly once -- no slot reuse, and every
        # matmul carries at most the single DMA wait walrus permits.
        cent_copies = []
        GRP = min(8, 2 * Bl)
        n_groups = (2 * Bl + GRP - 1) // GRP
        assert GRP % bpd == 0 and Bl % bpd == 0
        with (
            tc.tile_pool(name="big", bufs=bufs) as big_pool,
            tc.tile_pool(name="cps", bufs=8, space="PSUM") as cps,
        ):
            dma_engines = [nc.sync, nc.scalar, nc.gpsimd][:n_dma_eng]

            def emit_bulk():
                dma_i = 0
                for g in range(n_groups):
                    G = cps.tile([128, 512], F32, name="Gacc")
                    start_mm = None
                    col_last = []
                    for ti in range(GRP // bpd):
                        gi0 = g * GRP + ti * bpd
                        t_idx, b0 = divmod(gi0, Bl)
                        src = (e, v)[t_idx]
                        Tb = big_pool.tile([128, bpd * J * D], F32, name="Tb")
                        eng = dma_engines[dma_i % len(dma_engines)]
                        dma_i += 1
                        if bpd == 1:
                            eng.dma_start(
                                out=Tb[:],
                                in_=src[b0].rearrange("(p j) d -> p (j d)", p=128),
                            )
                        else:
                            eng.dma_start(
                                out=Tb[:].rearrange("p (b x) -> p b x", b=bpd),
                                in_=src[b0 : b0 + bpd].rearrange(
                                    "b (p j) d -> p b (j d)", p=128
                                ),
                            )
                        if reduce_mode == "dve":
                            # tree-halve the 16 seq rows per partition on DVE
                            # (exact fp32 adds) down to width dve_stop --
                            # one 3D-AP op covers all bpd batches in the tile
                            # to amortize DVE instruction overhead
                            w = J * D // 2
                            Tb3 = (
                                Tb[:].rearrange("p (b x) -> p b x", b=bpd)
                                if bpd > 1
                                else None
                            )
                            while w >= dve_stop:
                                if bpd == 1:
                                    nc.vector.tensor_add(
                                        Tb[:, 0:w],
                                        Tb[:, 0:w],
                                        Tb[:, w : 2 * w],
                                    )
                                else:
                                    nc.vector.tensor_add(
                                        Tb3[:, :, 0:w],
                                        Tb3[:, :, 0:w],
                                        Tb3[:, :, w : 2 * w],
                                    )
                                w //= 2
                        for bi in range(bpd):
                            c = ti * bpd + bi
                            if reduce_mode == "dve":
                                base = bi * J * D
                                n_folds = dve_stop // D
                                for fi in range(n_folds):
                                    is_first = c == 0 and fi == 0
                                    is_last = c == GRP - 1 and fi == n_folds - 1
                                    o = base + fi * D
                                    mm = nc.tensor.matmul(
                                        out=G[:, c : c + 1],
                                        lhsT=Tb[:, o : o + D],
                                        rhs=ones_col,
                                        start=is_first,
                                        stop=is_last,
                                    )
                                    if start_mm is None:
                                        start_mm = mm
                                    elif fi == 0:
                                        add_dep_helper(
                                            mm.ins, start_mm.ins, sync=False,
                                            reason="psum group start first",
                                        )
                                    if fi == n_folds - 1:
                                        col_last.append(mm)
                                    if is_last:
                                        for prev in col_last[:-1]:
                                            add_dep_helper(
                                                mm.ins, prev.ins, sync=False,
                                                reason="psum group stop last",
                                            )
                                continue
                            for j in range(0, J, pe_stride):
                                is_last = c == GRP - 1 and j + pe_stride >= J
                                o = (bi * J + j) * D
                                mm = nc.tensor.matmul(
                                    out=G[:, c : c + 1],
                                    lhsT=Tb[:, o : o + D],
                                    rhs=ones_col,
                                    start=(c == 0 and j == 0),
                                    stop=is_last,
                                )
                                if start_mm is None:
                                    start_mm = mm
                                elif j == 0:
                                    add_dep_helper(
                                        mm.ins, start_mm.ins, sync=False,
                                        reason="psum group start first",
                                    )
                                if j == J - 1:
                                    col_last.append(mm)
                                if is_last:
                                    for prev in col_last[:-1]:
                                        add_dep_helper(
                                            mm.ins, prev.ins, sync=False,
                                            reason="psum group stop last",
                                        )
                        start_mm = start_mm
                    cent_copies.append(
                        nc.scalar.mul(
                            centS[:, g * GRP : (g + 1) * GRP], G[:, 0:GRP], 1.0 / S
                        )
                    )

            if loop_r > 1:
                with tc.For_i(0, loop_r, 1) as _i:
                    emit_bulk()
            else:
                emit_bulk()

        # ---- gather centroids across cores ----
        nc.sync.dma_start(out=loc_cent[:], in_=centS[:])
        if solo:
            nc.sync.dma_start(out=gath[:], in_=loc_cent[:])
        else:
            nc.gpsimd.collective_compute(
                "AllGather",
                mybir.AluOpType.bypass,
                replica_groups=[list(range(n_cores))],
                ins=[loc_cent[:]],
                outs=[gath[:]],
            )
        CtEV = consts.tile([D, 2 * B], F32, name="CtEV")
        ct_dma = nc.sync.dma_start(
            out=CtEV[:].rearrange("p (t c j) -> p t c j", t=2, c=n_cores),
            in_=gath.rearrange("(c p) (t j) -> p t c j", c=n_cores, t=2),
        )
        CtE = CtEV[:, 0:B]
        CtV = CtEV[:, B : 2 * B]

        # ---- small pairwise phase (identical on every core) ----
        # PE gate: absorbs the cross-engine deps (cent copies on ACT, the
        # centroid reload DMA) so each later fp32 matmul needs <=1 sem wait
        # (walrus allows only one sync wait on an fp32 matmul's ldweights).
        pe_gate = nc.tensor.nop()
        for ins in cent_copies:
            add_dep_helper(pe_gate.ins, ins.ins, sync=True, reason="pe gate")
        add_dep_helper(pe_gate.ins, ct_dma.ins, sync=True, reason="pe gate dma")
        sp = ctx.enter_context(tc.tile_pool(name="sp", bufs=1))
        m2E = sp.tile([D, B], F32, name="m2E")
        m2V = sp.tile([D, B], F32, name="m2V")
        sqE = sp.tile([D, B], F32, name="sqE")
        sqV = sp.tile([D, B], F32, name="sqV")
        nc.vector.tensor_scalar_mul(m2E[:], CtE, -2.0)
        nc.vector.tensor_scalar_mul(m2V[:], CtV, -2.0)
        nc.vector.tensor_mul(sqE[:], CtE, CtE)
        nc.vector.tensor_mul(sqV[:], CtV, CtV)

        # aug tiles: row 0 and row 32 carry {norms, ones}; rest zero (K=64)
        ag_e = sp.tile([64, B], F32, name="ag_e")    # lhsT rows: n_e, 1
        ag_v = sp.tile([64, B], F32, name="ag_v")    # lhsT rows: n_v, 1
        rhs_e = sp.tile([64, B], F32, name="rhs_e")  # rhs rows: 1, n_e
        rhs_v = sp.tile([64, B], F32, name="rhs_v")  # rhs rows: 1, n_v
        acc = sp.tile([128, 3 * n_blk], F32, name="acc")  # per-partition partials
        nc.vector.memset(acc[:], 0.0)

        with tc.tile_pool(name="sps", bufs=1, space="PSUM") as sps, tc.tile_pool(
            name="spp", bufs=3 * n_blk, space="PSUM"
        ) as spp, tc.tile_pool(name="spf", bufs=1, space="PSUM") as spf, tc.tile_pool(
            name="trash", bufs=2
        ) as trash_pool:
            def pe_mm(*args, **kwargs):
                mm = nc.tensor.matmul(*args, **kwargs)
                add_dep_helper(mm.ins, pe_gate.ins, sync=False, reason="after gate")
                return mm

            psn = sps.tile([128, 512], F32, name="psn")
            # squared norms at partitions 0 and 32
            pe_mm(out=psn[0:1, 0:B], lhsT=ones_col, rhs=sqE[:])
            pe_mm(out=psn[32:33, 0:B], lhsT=ones_col, rhs=sqV[:])
            pe_mm(out=psn[0:1, B : 2 * B], lhsT=ones_col, rhs=sqV[:])
            pe_mm(out=psn[32:33, B : 2 * B], lhsT=ones_col, rhs=sqE[:])

            for t, row0_src, row32_src in (
                (ag_e, psn[0:1, 0:B], None),
                (ag_v, psn[0:1, B : 2 * B], None),
                (rhs_e, None, psn[32:33, B : 2 * B]),
                (rhs_v, None, psn[32:33, 0:B]),
            ):
                nc.vector.memset(t[:], 0.0)
                if row0_src is not None:
                    nc.vector.tensor_copy(t[0:1, :], row0_src)
                    nc.vector.memset(t[32:33, :], 1.0)
                else:
                    nc.vector.memset(t[0:1, :], 1.0)
                    nc.vector.tensor_copy(t[32:33, :], row32_src)

            # EV separation: sq = -2 E^T V + (n_e + n_v), then hinge
            for bi, (bs, bn) in enumerate(blks):
                P_evb = spp.tile([128, 512], F32, name="P_ev")
                P_ev = P_evb[:, 0:B]
                pe_mm(
                    out=P_ev[:bn],
                    lhsT=m2E[:, bs : bs + bn],
                    rhs=CtV,
                    start=True,
                    stop=False,
                )
                pe_mm(
                    out=P_ev[:bn],
                    lhsT=ag_e[:, bs : bs + bn],
                    rhs=rhs_v[:],
                    start=False,
                    stop=True,
                )
                dist = trash_pool.tile([128, B], F32, name="dist")
                hin = trash_pool.tile([128, B], F32, name="hin")
                hsq = trash_pool.tile([128, B], F32, name="hsq")
                nc.vector.tensor_scalar_max(P_ev[:bn], P_ev[:bn], 0.0)
                nc.scalar.activation(
                    dist[:bn], P_ev[:bn], mybir.ActivationFunctionType.Sqrt,
                    bias=b_eps[:bn],
                )
                nc.scalar.activation(
                    hin[:bn],
                    dist[:bn],
                    mybir.ActivationFunctionType.Relu,
                    bias=b_margin[:bn],
                    scale=-1.0,
                )
                nc.scalar.activation(
                    hsq[:bn],
                    hin[:bn],
                    mybir.ActivationFunctionType.Square,
                    accum_out=acc[:bn, bi : bi + 1],
                )

            # EE / VV clustering: sq = -2 X^T X + (n + n), relu, sum
            for pi, (m2, Ct, ag, rhs_t) in enumerate(
                ((m2E, CtE, ag_e, rhs_e), (m2V, CtV, ag_v, rhs_v))
            ):
                for bi, (bs, bn) in enumerate(blks):
                    P_clb = spp.tile([128, 512], F32, name="P_cl", tag="P_ev")
                    P_cl = P_clb[:, 0:B]
                    pe_mm(
                        out=P_cl[:bn],
                        lhsT=m2[:, bs : bs + bn],
                        rhs=Ct,
                        start=True,
                        stop=False,
                    )
                    pe_mm(
                        out=P_cl[:bn],
                        lhsT=ag[:, bs : bs + bn],
                        rhs=rhs_t[:],
                        start=False,
                        stop=True,
                    )
                    rel = trash_pool.tile([128, B], F32, name="rel", tag="dist")
                    ci = (1 + pi) * n_blk + bi
                    nc.scalar.activation(
                        rel[:bn],
                        P_cl[:bn],
                        mybir.ActivationFunctionType.Relu,
                        accum_out=acc[:bn, ci : ci + 1],
                    )

            # combine: tot = w_ev*sum(ev) + w_ee*sum(ee) + w_vv*sum(vv)
            t_ev = sp.tile([128, 1], F32, name="t_ev")
            t_ee = sp.tile([128, 1], F32, name="t_ee")
            t_vv = sp.tile([128, 1], F32, name="t_vv")
            tot = sp.tile([128, 1], F32, name="tot")
            for t, base in ((t_ev, 0), (t_ee, n_blk), (t_vv, 2 * n_blk)):
                if n_blk == 1:
                    nc.vector.tensor_copy(t[:], acc[:, base : base + 1])
                else:
                    nc.vector.tensor_add(
                        t[:], acc[:, base : base + 1], acc[:, base + 1 : base + 2]
                    )
                    for k in range(2, n_blk):
                        nc.vector.tensor_add(t[:], t[:], acc[:, base + k : base + k + 1])
            nc.vector.tensor_scalar_mul(tot[:], t_ev[:], w_ev)
            nc.vector.scalar_tensor_tensor(
                tot[:], t_ee[:], w_ee, tot[:],
                op0=mybir.AluOpType.mult, op1=mybir.AluOpType.add,
            )
            nc.vector.scalar_tensor_tensor(
                tot[:], t_vv[:], w_vv, tot[:],
                op0=mybir.AluOpType.mult, op1=mybir.AluOpType.add,
            )
            psFb = spf.tile([128, 512], F32, name="psF")
            psF = psFb[0:1, 0:1]
            pe_mm(out=psF, lhsT=ones_col, rhs=tot[:])
            fin = sp.tile([1, 1], F32, name="fin")
            if chain is not None:
                ch = sp.tile([1, 1], F32, name="ch")
                nc.sync.dma_start(out=ch[:], in_=chain[:])
                nc.vector.scalar_tensor_tensor(
                    fin[:], ch[:], 0.0, psF,
                    op0=mybir.AluOpType.mult, op1=mybir.AluOpType.add,
                )
            else:
                nc.scalar.copy(fin[:], psF)
            nc.sync.dma_start(out=out[:], in_=fin[:])


def build_nc(B=B, S=S, D=D, n_cores=N_CORES, with_chain=False, solo=False,
             bpd=None, bufs=None, loop_r=1, pe_stride=1, reduce_mode=None,
             dve_stop=None, n_dma_eng=None):
    Bl = B // n_cores
    nc = bacc.Bacc("TRN2", num_devices=n_cores)
    e = nc.dram_tensor("expert_concepts", [Bl, S, D], F32, kind="ExternalInput").ap()
    v = nc.dram_tensor("violator_concepts", [Bl, S, D], F32, kind="ExternalInput").ap()
    chain = (
        nc.dram_tensor("chain", [1, 1], F32, kind="ExternalInput").ap()
        if with_chain
        else None
    )
    out = nc.dram_tensor("out", [1, 1], F32, kind="ExternalOutput").ap()
    loc_cent = nc.dram_tensor("loc_cent", [D, 2 * Bl], F32).ap()
    gath_space = "Local" if solo else "Shared"
    gath = nc.dram_tensor(
        "gath", [n_cores * D, 2 * Bl], F32, addr_space=gath_space
    ).ap()
    with tile.TileContext(nc) as tc:
        _build_body(
            tc, e, v, out, loc_cent, gath, B, S, D, n_cores, chain=chain, solo=solo,
            bpd=bpd if bpd is not None else BPD,
            bufs=bufs if bufs is not None else BUFS,
            loop_r=loop_r,
            pe_stride=pe_stride,
            reduce_mode=reduce_mode if reduce_mode is not None else REDUCE_MODE,
            dve_stop=dve_stop if dve_stop is not None else DVE_STOP,
            n_dma_eng=n_dma_eng,
        )
    nc.compile()
    return nc


def _run(expert_concepts, violator_concepts, **spmd_kwargs):
    expert_concepts = np.ascontiguousarray(expert_concepts, dtype=np.float32)
    violator_concepts = np.ascontiguousarray(violator_concepts, dtype=np.float32)
    assert expert_concepts.shape == (B, S, D)
    assert violator_concepts.shape == (B, S, D)

    nc = build_nc()
    Bl = B // N_CORES
    in_maps = [
        {
            "expert_concepts": expert_concepts[c * Bl : (c + 1) * Bl],
            "violator_concepts": violator_concepts[c * Bl : (c + 1) * Bl],
        }
        for c in range(N_CORES)
    ]
    res = run_bass_kernel_spmd(nc, in_maps, list(range(N_CORES)), **spmd_kwargs)
    return np.float32(res.results[0]["out"][0, 0]), res


def kernel(expert_concepts: np.ndarray, violator_concepts: np.ndarray) -> np.ndarray:
    out, _ = _run(expert_concepts, violator_concepts)
    return out

